# revision 25
# baseline (speedup 1.0000x reference)
"""Shapelet distance transform kernel for Trainium2 (8 NeuronCores).

out[b, s] = min_w sum_{l,c} (data[b, w+l, c] - kernel[s, l, c])^2 / LS

Strategy (data-parallel over batch, 4 batches per core, no collectives):
PE emits per-window distances directly -- kaug rows are -2*K/LS for the
96 im2col rows (3 channels x 32 lags) plus 32 tap rows of 1/LS that
turn the staged d2 stream into the sliding norm a2/LS -- and the DVE
min-reduces each [128, 2048] (4-bank) PSUM tile to one column of a
mins tile; a final tiny reduce + k2/LS add per (batch, s-chunk)
produces the output.

Measured op-throughput on this HW pinned the drain design: every DVE
min-combining form (tensor_reduce, tensor_tensor, tensor_tensor_scan)
runs at ~1 elem/lane/cycle (tensor_tensor_reduce wedges the device;
GpSimd cannot run TensorTensor; no 2x DVE perf mode engages for min;
the scan's serial recurrence costs 2.16 cyc/step), so the drain floor
is ~68us/core on DVE and direct tensor_reduce from PSUM is optimal.
The win over the baseline is everything around it:

- The matmul rhs needs NO on-chip work at all: the host ships each
  batch as 4 de-interleaved streams xs[b] = [ch0 | ch1 | ch2 | d2]
  (pure layout transform + the 0.01%-of-FLOPs d2 stream), and an rhs
  tile [128 rows = (g, lag), 1024 windows] is ONE affine DMA: row
  g*32+l reads stream_g[w0+l .. w0+l+1024) t-contiguously. No PE
  transposes, no ACT im2col assembly, no PSUM->SBUF rhs eviction
  (baseline: 71us ACT + 24us PE transposes + 46us fp32r LDWEIGHTS).
- Loads go through the gpsimd software DGE: SWDGE swizzles descriptors
  across all 16 SDMA engines, where the HWDGE rings pinned these
  128-partition loads to 4 engines (measured ~100us wall for the 8MB).
- bf16 everywhere (rel err ~2.8e-3 vs the 2e-2 gate; fp8 would break
  the d2-tap accuracy, and DoubleRow fp8 cannot help a K=128
  contraction).

Window coverage: 8 groups of 1024 windows at starts {0, 1024, ...,
6144, 7137}; the odd-start last group exactly covers all 8161 windows
with a benign 31-window repeat (no phantom-window poisoning).

Engine budget per core (4 batches): DVE 32 reduces of 2048 + finals
~ 72us busy, gap-free (the floor); PE 128 matmuls of 512 bf16 ~ 46us;
ACT idle; DMA ~ 8.5MB spread over 16 SDMA engines; rhs pool bufs=3
rides out SDMA contention noise from co-tenants.
"""

import sys

for _p in ("/opt/trn_rl_repo",):
    if _p not in sys.path:
        sys.path.insert(0, _p)

from contextlib import ExitStack

import ml_dtypes
import numpy as np

import concourse.bacc as bacc
import concourse.bass as bass
import concourse.tile as tile
from concourse import mybir

F32 = mybir.dt.float32
BF16 = mybir.dt.bfloat16
AL = mybir.AluOpType
AF = mybir.ActivationFunctionType

B, T, C = 32, 8192, 3
NS, LS = 256, 32
W = T - LS + 1  # 8161 valid windows
NCORES = 8
BL = B // NCORES  # 4 batches per core
SCALE = 1.0 / LS
GROUPS = [0, 1024, 2048, 3072, 4096, 5120, 6144, 7137]  # 1024 windows each
NG = len(GROUPS)
SECT = 8192  # stream section per g (2 parities x 4096)
SBATCH = 4 * SECT  # xs elements per batch


def build_program() -> bass.Bass:
    nc = bacc.Bacc("TRN2", target_bir_lowering=False, debug=False)
    xs = [
        nc.dram_tensor(f"xs{b}", [SBATCH], BF16, kind="ExternalInput").ap()
        for b in range(BL)
    ]
    kaug = nc.dram_tensor("kaug", [2, 128, 128], BF16, kind="ExternalInput").ap()
    k2c = nc.dram_tensor("k2c", [2, 128], F32, kind="ExternalInput").ap()
    out = nc.dram_tensor("out", [BL, NS], F32, kind="ExternalOutput").ap()

    with tile.TileContext(nc) as tc, ExitStack() as ctx:
        consts = ctx.enter_context(tc.tile_pool(name="consts", bufs=1))
        kaug_sb = consts.tile([128, 2, 128], BF16)
        nc.sync.dma_start(
            out=kaug_sb,
            in_=bass.AP(
                tensor=kaug.tensor,
                offset=kaug.offset,
                ap=[[128, 128], [128 * 128, 2], [1, 128]],
            ),
        )
        k2sb = consts.tile([128, 2], F32)
        nc.sync.dma_start(
            out=k2sb,
            in_=bass.AP(
                tensor=k2c.tensor, offset=k2c.offset, ap=[[1, 128], [128, 2]]
            ),
        )

        # ---- Phase B: matmuls + direct min-reduce drain. ----
        rhs_pool = ctx.enter_context(tc.tile_pool(name="rhs", bufs=3))
        ps_pool = ctx.enter_context(tc.tile_pool(name="ps", bufs=2, space="PSUM"))
        mins_pool = ctx.enter_context(tc.tile_pool(name="mins", bufs=4))
        fin_pool = ctx.enter_context(tc.tile_pool(name="fin", bufs=8))

        def load_windows(tile_, b, w0, eng, n=1024):
            # rhs row p = g*32 + l holds stream_g[w0 + l + i], i in [0, n):
            # one t-contiguous DMA covers the whole tile. Loads go through
            # the gpsimd software DGE: SWDGE swizzles descriptors across all
            # 16 SDMA engines, where the HWDGE rings pinned these 128-
            # partition loads to 4 engines (measured ~100us wall for 8MB).
            eng.dma_start(
                out=tile_,
                in_=bass.AP(
                    tensor=xs[b].tensor,
                    offset=xs[b].offset + w0,
                    ap=[[SECT, 4], [1, 32], [1, n]],
                ),
            )

        for b in range(BL):
            rhs = []
            for gi, w0 in enumerate(GROUPS):
                rt = rhs_pool.tile([128, 1024], BF16, tag=f"g{gi}")
                # batch 0's first PSUM-pair tiles skip the Q7 emission queue
                # (HWDGE sync ring) so the PE/DVE pipeline starts earlier.
                eng = nc.sync if (b == 0 and gi < 2) else nc.gpsimd
                load_windows(rt, b, w0, eng)
                rhs.append(rt)
            mins0 = mins_pool.tile([128, NG // 2], F32, tag="m0")
            mins1 = mins_pool.tile([128, NG // 2], F32, tag="m1")
            mins = [mins0, mins1]
            for gp in range(NG // 2):  # 2 groups per PSUM tile (4 banks)
                for sc in range(2):
                    ps = ps_pool.tile([128, 2048], F32)
                    for gh in range(2):
                        gi = gp * 2 + gh
                        for h in range(2):
                            nc.tensor.matmul(
                                ps[:, gh * 1024 + h * 512 : gh * 1024 + (h + 1) * 512],
                                kaug_sb[:, sc, :],
                                rhs[gi][:, h * 512 : (h + 1) * 512],
                                start=True, stop=True,
                            )
                    nc.vector.tensor_reduce(
                        mins[sc][:, gp : gp + 1],
                        ps,
                        axis=mybir.AxisListType.X,
                        op=AL.min,
                    )
            for sc in range(2):
                res = fin_pool.tile([128, 1], F32, tag="res")
                nc.vector.tensor_reduce(
                    res, mins[sc], axis=mybir.AxisListType.X, op=AL.min
                )
                fin = fin_pool.tile([128, 1], F32, tag="fin")
                nc.vector.tensor_scalar(
                    out=fin,
                    in0=res,
                    scalar1=k2sb[:, sc : sc + 1],
                    scalar2=None,
                    op0=AL.add,
                )
                nc.sync.dma_start(
                    out=out[b, sc * 128 : (sc + 1) * 128].rearrange("(p o) -> p o", o=1),
                    in_=fin,
                )
    nc.compile()
    return nc


_PROGRAM = None


def _get_program() -> bass.Bass:
    global _PROGRAM
    if _PROGRAM is None:
        _PROGRAM = build_program()
    return _PROGRAM


def make_in_maps(data: np.ndarray, kernel: np.ndarray) -> list[dict]:
    assert data.shape == (B, T, C) and kernel.shape == (NS, LS, C)
    d32 = np.ascontiguousarray(data, dtype=np.float32)  # [B, T, C]
    xs_host = np.empty((B, 4, T), dtype=ml_dtypes.bfloat16)
    xs_host[:, :3, :] = d32.transpose(0, 2, 1)  # de-interleaved channels
    xs_host[:, 3, :] = (d32 * d32).sum(axis=2)  # d2 stream
    xs_host = xs_host.reshape(B, SBATCH)
    kb = np.ascontiguousarray(kernel, dtype=np.float32).astype(ml_dtypes.bfloat16)
    kf = kb.astype(np.float32)  # [NS, LS, C]
    kaug = np.zeros((2, 128, 128), dtype=np.float32)
    for sc in range(2):
        ks = kf[sc * 128 : (sc + 1) * 128]  # [128, LS, C]
        for c_ in range(C):
            for l in range(LS):
                kaug[sc, c_ * 32 + l, :] = -2.0 * SCALE * ks[:, l, c_]
        kaug[sc, 96:128, :] = SCALE  # d2 tap rows
    kaug = kaug.astype(ml_dtypes.bfloat16)
    k2 = ((kf * kf).sum(axis=(1, 2)) * SCALE).astype(np.float32)  # [NS]
    k2c = np.stack([k2[:128], k2[128:]]).astype(np.float32)  # [2, 128]
    maps = []
    for i in range(NCORES):
        m = {"kaug": kaug, "k2c": k2c}
        for b in range(BL):
            m[f"xs{b}"] = np.ascontiguousarray(xs_host[i * BL + b])
        maps.append(m)
    return maps


def kernel(data: np.ndarray, kernel: np.ndarray) -> np.ndarray:
    from concourse.bass_utils import run_bass_kernel_spmd

    in_maps = make_in_maps(data, kernel)
    nc = _get_program()
    res = run_bass_kernel_spmd(nc, in_maps, list(range(NCORES)))
    return np.concatenate(
        [res.results[i]["out"] for i in range(NCORES)], axis=0
    ).astype(np.float32)


# revision 26
# speedup vs baseline: 1.1571x; 1.1571x over previous
"""Shapelet distance transform kernel for Trainium2 (8 NeuronCores).

out[b, s] = min_w sum_{l,c} (data[b, w+l, c] - kernel[s, l, c])^2 / LS

Strategy (data-parallel over batch, 4 batches per core, no collectives):
PE emits per-window distances directly -- kaug rows are -2*K/LS for the
96 im2col rows (3 channels x 32 lags) plus 32 tap rows of 1/LS that
turn the staged d2 stream into the sliding norm a2/LS -- and the DVE
min-reduces each [128, 2048] (4-bank) PSUM tile to one column of a
mins tile; a final tiny reduce + k2/LS add per (batch, s-chunk)
produces the output.

Measured op-throughput on this HW pinned the drain design: every DVE
min-combining form (tensor_reduce, tensor_tensor, tensor_tensor_scan)
runs at ~1 elem/lane/cycle (tensor_tensor_reduce wedges the device;
GpSimd cannot run TensorTensor; no 2x DVE perf mode engages for min;
the scan's serial recurrence costs 2.16 cyc/step), so the drain floor
is ~68us/core on DVE and direct tensor_reduce from PSUM is optimal.
The win over the baseline is everything around it:

- The matmul rhs needs NO on-chip work at all: the host ships each
  batch as 4 de-interleaved streams xs[b] = [ch0 | ch1 | ch2 | d2]
  (pure layout transform + the 0.01%-of-FLOPs d2 stream), and an rhs
  tile [128 rows = (g, lag), 1024 windows] is ONE affine DMA: row
  g*32+l reads stream_g[w0+l .. w0+l+1024) t-contiguously. No PE
  transposes, no ACT im2col assembly, no PSUM->SBUF rhs eviction
  (baseline: 71us ACT + 24us PE transposes + 46us fp32r LDWEIGHTS).
- Loads go through the gpsimd software DGE: SWDGE swizzles descriptors
  across all 16 SDMA engines, where the HWDGE rings pinned these
  128-partition loads to 4 engines (measured ~100us wall for the 8MB).
- bf16 everywhere (rel err ~2.8e-3 vs the 2e-2 gate; fp8 would break
  the d2-tap accuracy, and DoubleRow fp8 cannot help a K=128
  contraction).

Window coverage: 8 groups of 1024 windows at starts {0, 1024, ...,
6144, 7137}; the odd-start last group exactly covers all 8161 windows
with a benign 31-window repeat (no phantom-window poisoning).

Engine budget per core (4 batches): DVE 32 reduces of 2048 + finals
~ 72us busy, gap-free (the floor); PE 128 matmuls of 512 bf16 ~ 46us;
ACT idle; DMA ~ 8.5MB spread over 16 SDMA engines; rhs pool bufs=3
rides out SDMA contention noise from co-tenants.
"""

import sys

for _p in ("/opt/trn_rl_repo",):
    if _p not in sys.path:
        sys.path.insert(0, _p)

from contextlib import ExitStack

import ml_dtypes
import numpy as np

import concourse.bacc as bacc
import concourse.bass as bass
import concourse.tile as tile
from concourse import mybir

F32 = mybir.dt.float32
BF16 = mybir.dt.bfloat16
AL = mybir.AluOpType
AF = mybir.ActivationFunctionType

B, T, C = 32, 8192, 3
NS, LS = 256, 32
W = T - LS + 1  # 8161 valid windows
NCORES = 8
BL = B // NCORES  # 4 batches per core
SCALE = 1.0 / LS
GROUPS = [0, 1024, 2048, 3072, 4096, 5120, 6144, 7137]  # 1024 windows each
NG = len(GROUPS)
SECT = 8192  # stream section per g (2 parities x 4096)
SBATCH = 4 * SECT  # xs elements per batch


def build_program() -> bass.Bass:
    nc = bacc.Bacc("TRN2", target_bir_lowering=False, debug=False)
    xs = [
        nc.dram_tensor(f"xs{b}", [SBATCH], BF16, kind="ExternalInput").ap()
        for b in range(BL)
    ]
    kaug = nc.dram_tensor("kaug", [2, 128, 128], BF16, kind="ExternalInput").ap()
    k2c = nc.dram_tensor("k2c", [2, 128], F32, kind="ExternalInput").ap()
    out = nc.dram_tensor("out", [BL, NS], F32, kind="ExternalOutput").ap()

    with tile.TileContext(nc) as tc, ExitStack() as ctx:
        consts = ctx.enter_context(tc.tile_pool(name="consts", bufs=1))
        kaug_sb = consts.tile([128, 2, 128], BF16)
        nc.sync.dma_start(
            out=kaug_sb,
            in_=bass.AP(
                tensor=kaug.tensor,
                offset=kaug.offset,
                ap=[[128, 128], [128 * 128, 2], [1, 128]],
            ),
        )
        k2sb = consts.tile([128, 2], F32)
        nc.sync.dma_start(
            out=k2sb,
            in_=bass.AP(
                tensor=k2c.tensor, offset=k2c.offset, ap=[[1, 128], [128, 2]]
            ),
        )

        # ---- Phase B: matmuls + direct min-reduce drain. ----
        rhs_pool = ctx.enter_context(tc.tile_pool(name="rhs", bufs=3))
        ps_pool = ctx.enter_context(tc.tile_pool(name="ps", bufs=2, space="PSUM"))
        mins_pool = ctx.enter_context(tc.tile_pool(name="mins", bufs=4))
        fin_pool = ctx.enter_context(tc.tile_pool(name="fin", bufs=8))

        def load_windows(tile_, b, w0, eng, n=1024):
            # rhs row p = g*32 + l holds stream_g[w0 + l + i], i in [0, n):
            # one t-contiguous DMA covers the whole tile. Loads go through
            # the gpsimd software DGE: SWDGE swizzles descriptors across all
            # 16 SDMA engines, where the HWDGE rings pinned these 128-
            # partition loads to 4 engines (measured ~100us wall for 8MB).
            eng.dma_start(
                out=tile_,
                in_=bass.AP(
                    tensor=xs[b].tensor,
                    offset=xs[b].offset + w0,
                    ap=[[SECT, 4], [1, 32], [1, n]],
                ),
            )

        for b in range(BL):
            rhs = []
            for gi, w0 in enumerate(GROUPS):
                rt = rhs_pool.tile([128, 1024], BF16, tag=f"g{gi}")
                load_windows(rt, b, w0, nc.gpsimd)
                rhs.append(rt)
            mins0 = mins_pool.tile([128, NG // 2], F32, tag="m0")
            mins1 = mins_pool.tile([128, NG // 2], F32, tag="m1")
            mins = [mins0, mins1]
            for gp in range(NG // 2):  # 2 groups per PSUM tile (4 banks)
                for sc in range(2):
                    ps = ps_pool.tile([128, 2048], F32)
                    for gh in range(2):
                        gi = gp * 2 + gh
                        for h in range(2):
                            nc.tensor.matmul(
                                ps[:, gh * 1024 + h * 512 : gh * 1024 + (h + 1) * 512],
                                kaug_sb[:, sc, :],
                                rhs[gi][:, h * 512 : (h + 1) * 512],
                                start=True, stop=True,
                            )
                    nc.vector.tensor_reduce(
                        mins[sc][:, gp : gp + 1],
                        ps,
                        axis=mybir.AxisListType.X,
                        op=AL.min,
                    )
            for sc in range(2):
                res = fin_pool.tile([128, 1], F32, tag="res")
                nc.vector.tensor_reduce(
                    res, mins[sc], axis=mybir.AxisListType.X, op=AL.min
                )
                fin = fin_pool.tile([128, 1], F32, tag="fin")
                nc.vector.tensor_scalar(
                    out=fin,
                    in0=res,
                    scalar1=k2sb[:, sc : sc + 1],
                    scalar2=None,
                    op0=AL.add,
                )
                nc.sync.dma_start(
                    out=out[b, sc * 128 : (sc + 1) * 128].rearrange("(p o) -> p o", o=1),
                    in_=fin,
                )
    nc.compile()
    return nc


_PROGRAM = None


def _get_program() -> bass.Bass:
    global _PROGRAM
    if _PROGRAM is None:
        _PROGRAM = build_program()
    return _PROGRAM


def make_in_maps(data: np.ndarray, kernel: np.ndarray) -> list[dict]:
    assert data.shape == (B, T, C) and kernel.shape == (NS, LS, C)
    d32 = np.ascontiguousarray(data, dtype=np.float32)  # [B, T, C]
    xs_host = np.empty((B, 4, T), dtype=ml_dtypes.bfloat16)
    xs_host[:, :3, :] = d32.transpose(0, 2, 1)  # de-interleaved channels
    xs_host[:, 3, :] = (d32 * d32).sum(axis=2)  # d2 stream
    xs_host = xs_host.reshape(B, SBATCH)
    kb = np.ascontiguousarray(kernel, dtype=np.float32).astype(ml_dtypes.bfloat16)
    kf = kb.astype(np.float32)  # [NS, LS, C]
    kaug = np.zeros((2, 128, 128), dtype=np.float32)
    for sc in range(2):
        ks = kf[sc * 128 : (sc + 1) * 128]  # [128, LS, C]
        for c_ in range(C):
            for l in range(LS):
                kaug[sc, c_ * 32 + l, :] = -2.0 * SCALE * ks[:, l, c_]
        kaug[sc, 96:128, :] = SCALE  # d2 tap rows
    kaug = kaug.astype(ml_dtypes.bfloat16)
    k2 = ((kf * kf).sum(axis=(1, 2)) * SCALE).astype(np.float32)  # [NS]
    k2c = np.stack([k2[:128], k2[128:]]).astype(np.float32)  # [2, 128]
    maps = []
    for i in range(NCORES):
        m = {"kaug": kaug, "k2c": k2c}
        for b in range(BL):
            m[f"xs{b}"] = np.ascontiguousarray(xs_host[i * BL + b])
        maps.append(m)
    return maps


def kernel(data: np.ndarray, kernel: np.ndarray) -> np.ndarray:
    from concourse.bass_utils import run_bass_kernel_spmd

    in_maps = make_in_maps(data, kernel)
    nc = _get_program()
    res = run_bass_kernel_spmd(nc, in_maps, list(range(NCORES)))
    return np.concatenate(
        [res.results[i]["out"] for i in range(NCORES)], axis=0
    ).astype(np.float32)
